# revision 13
# baseline (speedup 1.0000x reference)
"""Pipelined GEMM kernel for Trainium2, 8 NeuronCores.

Computes C = A @ B + ws*(ws+1)/2 with A:(8192,256) B:(256,8192) fp32.

Sharding: 2x4 grid over (M, N). Core (mi, ni) computes the
(4096, 2048) output block C[mi] x [ni]. No inter-core communication.

I/O precision: inputs are cast on the host to the PE's consumption
dtype (bf16, or fp8e4m3 with DoubleRow which doubles PE throughput),
and C is stored to HBM as bf16 and upcast to fp32 on the host. That
halves the kernel's HBM traffic vs fp32 I/O: per-core 16MB (C) + 3MB
(bf16 in) or 1.5MB (fp8 in). HW-measured floors on this box: store
path ~62us, bf16 PE ~78us (weights change every MM -> LDWEIGHTS not
hidden + HAM), fp8 or weight-reuse orderings lower.

Per-core kernel (Tile framework):
  - A^T shard staged K-major (contraction dim on partitions); B
    likewise; fp8 uses the DoubleRow 3D layout [128, 2, X] (2 k-values
    per partition). Heads of both are loaded first so the first m-tiles
    can start after ~0.5MB.
  - Main loop over 32 m-tiles; stationary weights are reused across all
    4 n-chunks of an m-tile (LDWEIGHTS once per (m,k) instead of per
    MM). +const is fused into the PSUM->SBUF copyback, split between
    DVE and ACT so each PSUM tile frees fast.
  - One 0.5MB store DMA per m-tile, alternating between the two HWDGE
    rings (sync / scalar); the last m-tile is split into 2 x 0.25MB
    pieces on both rings to shorten the serial tail.
"""

import contextlib

import ml_dtypes
import numpy as np

import concourse.mybir as mybir
import concourse.tile as tile
from concourse import bacc
from concourse.bass_utils import run_bass_kernel_spmd

M, K, N = 8192, 256, 8192
NCORES = 8
RM, RN = 2, 4  # core grid over (M, N)
MS = M // RM  # 4096 rows of C per core
NS = N // RN  # 2048 cols of C per core
P = 128
MT = MS // P  # 32 m-tiles
KT = K // P  # 2 k-tiles
NCHUNK = 512  # one fp32 PSUM bank / max matmul free dim
NT = NS // NCHUNK  # 4 n-chunks = one [128, 2048] output tile per m-tile

F32 = mybir.dt.float32
BF16 = mybir.dt.bfloat16
FP8 = mybir.dt.float8e4

USE_FP8 = True  # production-path selector for kernel()


def build_program(const_add: float, repeat: int = 1, loop_opts: dict | None = None,
                  tail_split: bool = True, timing: bool = False,
                  probe: str | None = None, fp8: bool = False,
                  wreuse: bool = True):
    """repeat>1 wraps the whole body in a HW loop - used only by the
    timing harness. timing=True additionally makes `c` an Internal DRAM
    scratch tensor and adds a tiny dummy ExternalOutput, so a timing
    execution doesn't ship 32MB/core of outputs over the axon tunnel
    (the kernel's DMA work is unchanged).

    probe selects a stripped variant for HW bottleneck isolation:
      "pe"    - loads + matmuls only
      "copy"  - loads + matmuls + copybacks, no stores
      "dma"   - loads + half-matmuls + copybacks + stores
      "store" - loads + stores only

    fp8: inputs are fp8e4m3 in the DoubleRow 3D layout; one DoubleRow
    MM does the full K=256 contraction at 2 rows/cycle.
    wreuse (bf16 only): k-outer MM ordering so the stationary weights
    are loaded once per (m, k) and stream all 4 n-chunks.
    """
    do_cb = probe in (None, "copy", "dma")
    do_st = probe in (None, "dma", "store")
    do_mm = probe in (None, "pe", "copy", "dma")
    kt_eff = 1 if probe == "dma" else KT

    nc = bacc.Bacc("TRN2", target_bir_lowering=False, debug=False)
    in_dt = FP8 if fp8 else BF16
    if fp8:
        at = nc.dram_tensor("at", [P, KT, MS], FP8, kind="ExternalInput")
        b = nc.dram_tensor("b", [P, KT, NS], FP8, kind="ExternalInput")
    else:
        at = nc.dram_tensor("at", [K, MS], BF16, kind="ExternalInput")
        b = nc.dram_tensor("b", [K, NS], BF16, kind="ExternalInput")
    c_kind = "Internal" if timing else "ExternalOutput"
    c = nc.dram_tensor("c", [MS, NS], BF16, kind=c_kind)
    dummy = (nc.dram_tensor("tout", [P, 16], in_dt, kind="ExternalOutput")
             if timing else None)

    psum_bufs = 2 if (wreuse and not fp8) else 4
    with tile.TileContext(nc) as tc:
        with (
            tc.tile_pool(name="bpool", bufs=1) as bpool,
            tc.tile_pool(name="atpool", bufs=1) as atpool,
            tc.tile_pool(name="psum", bufs=psum_bufs, space="PSUM") as psum_pool,
            tc.tile_pool(name="opool", bufs=4) as opool,
            tc.For_i(0, repeat, 1, **(loop_opts or {}))
            if repeat > 1 else contextlib.nullcontext(),
        ):
            if fp8:
                at_t = atpool.tile([P, KT, MS], FP8, name="at_t", tag="at")
                b_t = bpool.tile([P, KT, NS], FP8, name="b_t", tag="b")
                at_dsts = [lambda c0, w: at_t[:, :, c0 : c0 + w]]
                b_dsts = [lambda c0, w: b_t[:, :, c0 : c0 + w]]
                at_srcs = [lambda c0, w: at[:, :, c0 : c0 + w]]
                b_srcs = [lambda c0, w: b[:, :, c0 : c0 + w]]
            else:
                at_sb = [
                    atpool.tile([P, MS], BF16, name=f"at{k}", tag=f"at{k}")
                    for k in range(KT)
                ]
                b_sb = [
                    bpool.tile([P, NS], BF16, name=f"b{k}", tag=f"b{k}")
                    for k in range(KT)
                ]
                at_dsts = [
                    (lambda k: lambda c0, w: at_sb[k][:, c0 : c0 + w])(k)
                    for k in range(KT)
                ]
                b_dsts = [
                    (lambda k: lambda c0, w: b_sb[k][:, c0 : c0 + w])(k)
                    for k in range(KT)
                ]
                at_srcs = [
                    (lambda k: lambda c0, w: at[k * P : (k + 1) * P,
                                               c0 : c0 + w])(k)
                    for k in range(KT)
                ]
                b_srcs = [
                    (lambda k: lambda c0, w: b[k * P : (k + 1) * P,
                                              c0 : c0 + w])(k)
                    for k in range(KT)
                ]

            # Loads: heads of A^T and B first (first m-tiles' operands),
            # then the rests. Alternate the two HWDGE rings.
            AHEAD = 512
            BHEAD = 512
            pieces = []
            for d, s in zip(at_dsts, at_srcs):
                pieces.append((d, s, 0, AHEAD))
            for d, s in zip(b_dsts, b_srcs):
                pieces.append((d, s, 0, BHEAD))
            for d, s in zip(b_dsts, b_srcs):
                pieces.append((d, s, BHEAD, NS - BHEAD))
            half = (MS - AHEAD) // 2
            for d, s in zip(at_dsts, at_srcs):
                pieces.append((d, s, AHEAD, half))
            for d, s in zip(at_dsts, at_srcs):
                pieces.append((d, s, AHEAD + half, MS - AHEAD - half))
            for i, (dst, src, c0, w) in enumerate(pieces):
                eng = nc.sync if i % 2 == 0 else nc.scalar
                eng.dma_start(dst(c0, w), src(c0, w))

            # Main GEMM loop; one 0.5MB store DMA per m-tile.
            for m in range(MT):
                ot = None
                if do_cb:
                    ot = opool.tile([P, NS], BF16, name="ot")
                if do_mm and fp8:
                    for jj in range(NT // 2):
                        ps = psum_pool.tile([P, 2 * NCHUNK], F32, name="ps")
                        for j2 in range(2):
                            jc = jj * 2 + j2
                            nc.tensor.matmul(
                                ps[:, j2 * NCHUNK : (j2 + 1) * NCHUNK],
                                at_t[:, :, m * P : (m + 1) * P],
                                b_t[:, :, jc * NCHUNK : (jc + 1) * NCHUNK],
                                start=True,
                                stop=True,
                                perf_mode=mybir.MatmulPerfMode.DoubleRow,
                            )
                        if do_cb:
                            col = jj * 2 * NCHUNK
                            nc.vector.tensor_scalar_add(
                                ot[:, col : col + NCHUNK], ps[:, :NCHUNK],
                                const_add)
                            nc.scalar.activation(
                                ot[:, col + NCHUNK : col + 2 * NCHUNK],
                                ps[:, NCHUNK:],
                                mybir.ActivationFunctionType.Copy,
                                bias=const_add,
                            )
                elif do_mm and wreuse:
                    # k-outer: LDWEIGHTS once per (m, k), 4 MMs stream.
                    ps = psum_pool.tile([P, NS], F32, name="ps")
                    for k in range(kt_eff):
                        for jc in range(NT):
                            nc.tensor.matmul(
                                ps[:, jc * NCHUNK : (jc + 1) * NCHUNK],
                                at_sb[k][:, m * P : (m + 1) * P],
                                b_sb[k][:, jc * NCHUNK : (jc + 1) * NCHUNK],
                                start=(k == 0),
                                stop=(k == kt_eff - 1),
                            )
                    if do_cb:
                        for jc in range(NT):
                            col = jc * NCHUNK
                            if jc % 2 == 0:
                                nc.vector.tensor_scalar_add(
                                    ot[:, col : col + NCHUNK],
                                    ps[:, col : col + NCHUNK], const_add)
                            else:
                                nc.scalar.activation(
                                    ot[:, col : col + NCHUNK],
                                    ps[:, col : col + NCHUNK],
                                    mybir.ActivationFunctionType.Copy,
                                    bias=const_add,
                                )
                elif do_mm:
                    for jj in range(NT // 2):
                        ps = psum_pool.tile([P, 2 * NCHUNK], F32, name="ps")
                        for j2 in range(2):
                            jc = jj * 2 + j2
                            for k in range(kt_eff):
                                nc.tensor.matmul(
                                    ps[:, j2 * NCHUNK : (j2 + 1) * NCHUNK],
                                    at_sb[k][:, m * P : (m + 1) * P],
                                    b_sb[k][:, jc * NCHUNK : (jc + 1) * NCHUNK],
                                    start=(k == 0),
                                    stop=(k == kt_eff - 1),
                                )
                        if do_cb:
                            col = jj * 2 * NCHUNK
                            nc.vector.tensor_scalar_add(
                                ot[:, col : col + NCHUNK], ps[:, :NCHUNK],
                                const_add)
                            nc.scalar.activation(
                                ot[:, col + NCHUNK : col + 2 * NCHUNK],
                                ps[:, NCHUNK:],
                                mybir.ActivationFunctionType.Copy,
                                bias=const_add,
                            )

                if not do_st:
                    continue
                src = ot if ot is not None else (
                    b_t[:, 0, :] if fp8 else b_sb[m % KT])
                if m < MT - 1 or not tail_split:
                    dma_eng = nc.sync if m % 2 == 0 else nc.scalar
                    dma_eng.dma_start(c[m * P : (m + 1) * P, :], src[:])
                else:
                    for nh in range(2):
                        dma_eng = nc.sync if nh % 2 == 0 else nc.scalar
                        dma_eng.dma_start(
                            c[m * P : (m + 1) * P,
                              nh * (NS // 2) : (nh + 1) * (NS // 2)],
                            src[:, nh * (NS // 2) : (nh + 1) * (NS // 2)],
                        )
            if dummy is not None:
                if fp8:
                    nc.sync.dma_start(dummy[:], b_t[:, 0, :16])
                else:
                    nc.sync.dma_start(dummy[:], b_sb[0][:, :16])

    nc.compile()
    return nc


_CACHE = {}


def _get_program(const_add: float):
    key = (const_add, USE_FP8)
    if key not in _CACHE:
        _CACHE[key] = build_program(const_add, fp8=USE_FP8)
    return _CACHE[key]


def make_in_maps(A, B, fp8: bool = False):
    """2x4 (M, N) grid; A shards staged K-major; both inputs cast on the
    host to the PE dtype. fp8 uses the DoubleRow [128, 2, X] layout
    (k = s*128 + p)."""
    if fp8:
        f8 = ml_dtypes.float8_e4m3
        A8 = np.asarray(A, dtype=f8)
        B8 = np.asarray(B, dtype=f8)
        maps = []
        for i in range(NCORES):
            mi, ni = divmod(i, RN)
            at = A8[mi * MS : (mi + 1) * MS].T  # [K, MS]
            bb = B8[:, ni * NS : (ni + 1) * NS]  # [K, NS]
            maps.append({
                "at": np.ascontiguousarray(
                    at.reshape(KT, P, MS).transpose(1, 0, 2)),
                "b": np.ascontiguousarray(
                    bb.reshape(KT, P, NS).transpose(1, 0, 2)),
            })
        return maps
    A16 = np.asarray(A, dtype=ml_dtypes.bfloat16)
    B16 = np.asarray(B, dtype=ml_dtypes.bfloat16)
    maps = []
    for i in range(NCORES):
        mi, ni = divmod(i, RN)
        maps.append({
            "at": np.ascontiguousarray(A16[mi * MS : (mi + 1) * MS].T),
            "b": np.ascontiguousarray(B16[:, ni * NS : (ni + 1) * NS]),
        })
    return maps


def assemble(results):
    rows = []
    for mi in range(RM):
        rows.append(np.concatenate(
            [np.asarray(results[mi * RN + ni]["c"], dtype=np.float32)
             for ni in range(RN)], axis=1))
    return np.concatenate(rows, axis=0)


def run(A, B, world_size, trace=False, **spmd_kwargs):
    A = np.ascontiguousarray(np.asarray(A, dtype=np.float32))
    B = np.ascontiguousarray(np.asarray(B, dtype=np.float32))
    ws = int(world_size)
    const_add = float(ws * (ws + 1) / 2)
    assert A.shape == (M, K) and B.shape == (K, N)

    nc = _get_program(const_add)
    res = run_bass_kernel_spmd(
        nc, make_in_maps(A, B, fp8=USE_FP8), list(range(NCORES)),
        trace=trace, **spmd_kwargs
    )
    return assemble(res.results), res


def kernel(A, B, world_size, **_unused):
    out, _ = run(A, B, world_size, trace=False)
    return out


# revision 14
# speedup vs baseline: 1.9381x; 1.9381x over previous
"""Pipelined GEMM kernel for Trainium2, 8 NeuronCores.

Computes C = A @ B + ws*(ws+1)/2 with A:(8192,256) B:(256,8192) fp32.

Sharding: 2x4 grid over (M, N). Core (mi, ni) computes the
(4096, 2048) output block C[mi] x [ni]. No inter-core communication.

I/O precision: inputs are cast on the host to the PE's consumption
dtype (bf16, or fp8e4m3 with DoubleRow which doubles PE throughput),
and C is stored to HBM as bf16 and upcast to fp32 on the host. That
halves the kernel's HBM traffic vs fp32 I/O: per-core 16MB (C) + 3MB
(bf16 in) or 1.5MB (fp8 in). HW-measured floors on this box: store
path ~62us, bf16 PE ~78us (weights change every MM -> LDWEIGHTS not
hidden + HAM), fp8 or weight-reuse orderings lower.

Per-core kernel (Tile framework):
  - A^T shard staged K-major (contraction dim on partitions); B
    likewise; fp8 uses the DoubleRow 3D layout [128, 2, X] (2 k-values
    per partition). Heads of both are loaded first so the first m-tiles
    can start after ~0.5MB.
  - Main loop over 32 m-tiles; stationary weights are reused across all
    4 n-chunks of an m-tile (LDWEIGHTS once per (m,k) instead of per
    MM). +const is fused into the PSUM->SBUF copyback, split between
    DVE and ACT so each PSUM tile frees fast.
  - One 0.5MB store DMA per m-tile, alternating between the two HWDGE
    rings (sync / scalar); the last m-tile is split into 2 x 0.25MB
    pieces on both rings to shorten the serial tail.
"""

import contextlib

import ml_dtypes
import numpy as np

import concourse.mybir as mybir
import concourse.tile as tile
from concourse import bacc
from concourse.bass_utils import run_bass_kernel_spmd

M, K, N = 8192, 256, 8192
NCORES = 8
RM, RN = 2, 4  # core grid over (M, N)
MS = M // RM  # 4096 rows of C per core
NS = N // RN  # 2048 cols of C per core
P = 128
MT = MS // P  # 32 m-tiles
KT = K // P  # 2 k-tiles
NCHUNK = 512  # one fp32 PSUM bank / max matmul free dim
NT = NS // NCHUNK  # 4 n-chunks = one [128, 2048] output tile per m-tile

F32 = mybir.dt.float32
BF16 = mybir.dt.bfloat16
FP8 = mybir.dt.float8e4

USE_FP8 = True  # production-path selector for kernel()


def build_program(const_add: float, repeat: int = 1, loop_opts: dict | None = None,
                  tail_split: bool = True, timing: bool = False,
                  probe: str | None = None, fp8: bool = False,
                  wreuse: bool = False):
    """repeat>1 wraps the whole body in a HW loop - used only by the
    timing harness. timing=True additionally makes `c` an Internal DRAM
    scratch tensor and adds a tiny dummy ExternalOutput, so a timing
    execution doesn't ship 32MB/core of outputs over the axon tunnel
    (the kernel's DMA work is unchanged).

    probe selects a stripped variant for HW bottleneck isolation:
      "pe"    - loads + matmuls only
      "copy"  - loads + matmuls + copybacks, no stores
      "dma"   - loads + half-matmuls + copybacks + stores
      "store" - loads + stores only

    fp8: inputs are fp8e4m3 in the DoubleRow 3D layout; one DoubleRow
    MM does the full K=256 contraction at 2 rows/cycle.
    wreuse (bf16 only): k-outer MM ordering so the stationary weights
    are loaded once per (m, k) and stream all 4 n-chunks.
    """
    do_cb = probe in (None, "copy", "dma")
    do_st = probe in (None, "dma", "store")
    do_mm = probe in (None, "pe", "copy", "dma")
    kt_eff = 1 if probe == "dma" else KT

    nc = bacc.Bacc("TRN2", target_bir_lowering=False, debug=False)
    in_dt = FP8 if fp8 else BF16
    if fp8:
        at = nc.dram_tensor("at", [P, KT, MS], FP8, kind="ExternalInput")
        b = nc.dram_tensor("b", [P, KT, NS], FP8, kind="ExternalInput")
    else:
        at = nc.dram_tensor("at", [K, MS], BF16, kind="ExternalInput")
        b = nc.dram_tensor("b", [K, NS], BF16, kind="ExternalInput")
    c_kind = "Internal" if timing else "ExternalOutput"
    c = nc.dram_tensor("c", [MS, NS], BF16, kind=c_kind)
    dummy = (nc.dram_tensor("tout", [P, 16], in_dt, kind="ExternalOutput")
             if timing else None)

    psum_bufs = 2 if (wreuse and not fp8) else 4
    with tile.TileContext(nc) as tc:
        with (
            tc.tile_pool(name="bpool", bufs=1) as bpool,
            tc.tile_pool(name="atpool", bufs=1) as atpool,
            tc.tile_pool(name="psum", bufs=psum_bufs, space="PSUM") as psum_pool,
            tc.tile_pool(name="opool", bufs=4) as opool,
            tc.For_i(0, repeat, 1, **(loop_opts or {}))
            if repeat > 1 else contextlib.nullcontext(),
        ):
            if fp8:
                at_t = atpool.tile([P, KT, MS], FP8, name="at_t", tag="at")
                b_t = bpool.tile([P, KT, NS], FP8, name="b_t", tag="b")
                at_dsts = [lambda c0, w: at_t[:, :, c0 : c0 + w]]
                b_dsts = [lambda c0, w: b_t[:, :, c0 : c0 + w]]
                at_srcs = [lambda c0, w: at[:, :, c0 : c0 + w]]
                b_srcs = [lambda c0, w: b[:, :, c0 : c0 + w]]
            else:
                at_sb = [
                    atpool.tile([P, MS], BF16, name=f"at{k}", tag=f"at{k}")
                    for k in range(KT)
                ]
                b_sb = [
                    bpool.tile([P, NS], BF16, name=f"b{k}", tag=f"b{k}")
                    for k in range(KT)
                ]
                at_dsts = [
                    (lambda k: lambda c0, w: at_sb[k][:, c0 : c0 + w])(k)
                    for k in range(KT)
                ]
                b_dsts = [
                    (lambda k: lambda c0, w: b_sb[k][:, c0 : c0 + w])(k)
                    for k in range(KT)
                ]
                at_srcs = [
                    (lambda k: lambda c0, w: at[k * P : (k + 1) * P,
                                               c0 : c0 + w])(k)
                    for k in range(KT)
                ]
                b_srcs = [
                    (lambda k: lambda c0, w: b[k * P : (k + 1) * P,
                                              c0 : c0 + w])(k)
                    for k in range(KT)
                ]

            # Loads: heads of A^T and B first (first m-tiles' operands),
            # then the rests. Alternate the two HWDGE rings.
            AHEAD = 512
            BHEAD = 512
            pieces = []
            for d, s in zip(at_dsts, at_srcs):
                pieces.append((d, s, 0, AHEAD))
            for d, s in zip(b_dsts, b_srcs):
                pieces.append((d, s, 0, BHEAD))
            for d, s in zip(b_dsts, b_srcs):
                pieces.append((d, s, BHEAD, NS - BHEAD))
            half = (MS - AHEAD) // 2
            for d, s in zip(at_dsts, at_srcs):
                pieces.append((d, s, AHEAD, half))
            for d, s in zip(at_dsts, at_srcs):
                pieces.append((d, s, AHEAD + half, MS - AHEAD - half))
            for i, (dst, src, c0, w) in enumerate(pieces):
                eng = nc.sync if i % 2 == 0 else nc.scalar
                eng.dma_start(dst(c0, w), src(c0, w))

            # Main GEMM loop; one 0.5MB store DMA per m-tile.
            for m in range(MT):
                ot = None
                if do_cb:
                    ot = opool.tile([P, NS], BF16, name="ot")
                if do_mm and fp8:
                    for jj in range(NT // 2):
                        ps = psum_pool.tile([P, 2 * NCHUNK], F32, name="ps")
                        for j2 in range(2):
                            jc = jj * 2 + j2
                            nc.tensor.matmul(
                                ps[:, j2 * NCHUNK : (j2 + 1) * NCHUNK],
                                at_t[:, :, m * P : (m + 1) * P],
                                b_t[:, :, jc * NCHUNK : (jc + 1) * NCHUNK],
                                start=True,
                                stop=True,
                                perf_mode=mybir.MatmulPerfMode.DoubleRow,
                            )
                        if do_cb:
                            col = jj * 2 * NCHUNK
                            nc.vector.tensor_scalar_add(
                                ot[:, col : col + NCHUNK], ps[:, :NCHUNK],
                                const_add)
                            nc.scalar.activation(
                                ot[:, col + NCHUNK : col + 2 * NCHUNK],
                                ps[:, NCHUNK:],
                                mybir.ActivationFunctionType.Copy,
                                bias=const_add,
                            )
                elif do_mm and wreuse:
                    # k-outer: LDWEIGHTS once per (m, k), 4 MMs stream.
                    ps = psum_pool.tile([P, NS], F32, name="ps")
                    for k in range(kt_eff):
                        for jc in range(NT):
                            nc.tensor.matmul(
                                ps[:, jc * NCHUNK : (jc + 1) * NCHUNK],
                                at_sb[k][:, m * P : (m + 1) * P],
                                b_sb[k][:, jc * NCHUNK : (jc + 1) * NCHUNK],
                                start=(k == 0),
                                stop=(k == kt_eff - 1),
                            )
                    if do_cb:
                        for jc in range(NT):
                            col = jc * NCHUNK
                            if jc % 2 == 0:
                                nc.vector.tensor_scalar_add(
                                    ot[:, col : col + NCHUNK],
                                    ps[:, col : col + NCHUNK], const_add)
                            else:
                                nc.scalar.activation(
                                    ot[:, col : col + NCHUNK],
                                    ps[:, col : col + NCHUNK],
                                    mybir.ActivationFunctionType.Copy,
                                    bias=const_add,
                                )
                elif do_mm:
                    for jj in range(NT // 2):
                        ps = psum_pool.tile([P, 2 * NCHUNK], F32, name="ps")
                        for j2 in range(2):
                            jc = jj * 2 + j2
                            for k in range(kt_eff):
                                nc.tensor.matmul(
                                    ps[:, j2 * NCHUNK : (j2 + 1) * NCHUNK],
                                    at_sb[k][:, m * P : (m + 1) * P],
                                    b_sb[k][:, jc * NCHUNK : (jc + 1) * NCHUNK],
                                    start=(k == 0),
                                    stop=(k == kt_eff - 1),
                                )
                        if do_cb:
                            col = jj * 2 * NCHUNK
                            nc.vector.tensor_scalar_add(
                                ot[:, col : col + NCHUNK], ps[:, :NCHUNK],
                                const_add)
                            nc.scalar.activation(
                                ot[:, col + NCHUNK : col + 2 * NCHUNK],
                                ps[:, NCHUNK:],
                                mybir.ActivationFunctionType.Copy,
                                bias=const_add,
                            )

                if not do_st:
                    continue
                src = ot if ot is not None else (
                    b_t[:, 0, :] if fp8 else b_sb[m % KT])
                if m < MT - 1 or not tail_split:
                    dma_eng = nc.sync if m % 2 == 0 else nc.scalar
                    dma_eng.dma_start(c[m * P : (m + 1) * P, :], src[:])
                else:
                    for nh in range(2):
                        dma_eng = nc.sync if nh % 2 == 0 else nc.scalar
                        dma_eng.dma_start(
                            c[m * P : (m + 1) * P,
                              nh * (NS // 2) : (nh + 1) * (NS // 2)],
                            src[:, nh * (NS // 2) : (nh + 1) * (NS // 2)],
                        )
            if dummy is not None:
                if fp8:
                    nc.sync.dma_start(dummy[:], b_t[:, 0, :16])
                else:
                    nc.sync.dma_start(dummy[:], b_sb[0][:, :16])

    nc.compile()
    return nc


_CACHE = {}


def _get_program(const_add: float):
    key = (const_add, USE_FP8)
    if key not in _CACHE:
        _CACHE[key] = build_program(const_add, fp8=USE_FP8)
    return _CACHE[key]


def make_in_maps(A, B, fp8: bool = False):
    """2x4 (M, N) grid; A shards staged K-major; both inputs cast on the
    host to the PE dtype. fp8 uses the DoubleRow [128, 2, X] layout
    (k = s*128 + p)."""
    if fp8:
        f8 = ml_dtypes.float8_e4m3
        A8 = np.asarray(A, dtype=f8)
        B8 = np.asarray(B, dtype=f8)
        maps = []
        for i in range(NCORES):
            mi, ni = divmod(i, RN)
            at = A8[mi * MS : (mi + 1) * MS].T  # [K, MS]
            bb = B8[:, ni * NS : (ni + 1) * NS]  # [K, NS]
            maps.append({
                "at": np.ascontiguousarray(
                    at.reshape(KT, P, MS).transpose(1, 0, 2)),
                "b": np.ascontiguousarray(
                    bb.reshape(KT, P, NS).transpose(1, 0, 2)),
            })
        return maps
    A16 = np.asarray(A, dtype=ml_dtypes.bfloat16)
    B16 = np.asarray(B, dtype=ml_dtypes.bfloat16)
    maps = []
    for i in range(NCORES):
        mi, ni = divmod(i, RN)
        maps.append({
            "at": np.ascontiguousarray(A16[mi * MS : (mi + 1) * MS].T),
            "b": np.ascontiguousarray(B16[:, ni * NS : (ni + 1) * NS]),
        })
    return maps


def assemble(results):
    rows = []
    for mi in range(RM):
        rows.append(np.concatenate(
            [np.asarray(results[mi * RN + ni]["c"], dtype=np.float32)
             for ni in range(RN)], axis=1))
    return np.concatenate(rows, axis=0)


def run(A, B, world_size, trace=False, **spmd_kwargs):
    A = np.ascontiguousarray(np.asarray(A, dtype=np.float32))
    B = np.ascontiguousarray(np.asarray(B, dtype=np.float32))
    ws = int(world_size)
    const_add = float(ws * (ws + 1) / 2)
    assert A.shape == (M, K) and B.shape == (K, N)

    nc = _get_program(const_add)
    res = run_bass_kernel_spmd(
        nc, make_in_maps(A, B, fp8=USE_FP8), list(range(NCORES)),
        trace=trace, **spmd_kwargs
    )
    return assemble(res.results), res


def kernel(A, B, world_size, **_unused):
    out, _ = run(A, B, world_size, trace=False)
    return out
